# revision 1
# baseline (speedup 1.0000x reference)
"""Trainium2 Bass kernel for complex-valued channel attention (XCA-style) — v2.

Same algorithm/sharding as the baseline (see kernel.py docstring), with the
PE diet rebuilt around dtype tiering:

  - v-path conv: 9 fused dense taps in bf16 (1.0 cyc/row, same rate as the
    f32r baseline but half the SBUF/DMA).
  - q,k-path conv: fp8(e4m3) with DoubleRow perf mode.  Taps are paired:
    one DR matmul contracts two taps at once at 0.5 cyc/row.  The rhs for a
    pair is a single overlapping strided AP [128, 2(tap delta), 258(flat
    padded row)] — the pair dim's stride is the element delta between the
    two taps' input windows, so no data is duplicated.  A PSUM bank holds
    one 258-wide output row (bank limit is 512 f32), so each tap matmul is
    issued per w-row; weights are stationary across the two rows.
    q,k quantization noise washes out in the attention logits (summed over
    65536 tokens), so fp8 costs ~nothing in final accuracy.
  - q-norms come from an ACT Square+accum pass (the QQ^T matmul is gone).
  - QK^T: fp8 DoubleRow too — transposed token blocks pair up naturally
    ([128, 2(block), 128] APs), contracting 256 tokens per matmul.
  - attn@v + projection: bf16.
  - y writeback: bf16 (host upcasts), halving output DMA.
"""

import sys

sys.path.insert(0, '/opt/trn_rl_repo')

import numpy as np

import concourse.bass as bass  # noqa: F401  (registers bass types)
import concourse.tile as tile
from concourse import bacc, mybir
from concourse.ap import AP
from concourse.bass_utils import run_bass_kernel_spmd
from concourse.masks import make_identity

F32 = mybir.dt.float32
BF16 = mybir.dt.bfloat16
FP8 = mybir.dt.float8e4

B, C, W, H = 2, 64, 256, 256
NCORES = 8
WL = W // NCORES          # 32 local w rows per core
HP = H + 2                # 258: h with zero pad columns
WLH = WL + 2              # 34: local w rows + halo
NWT = WL // 2             # 16 tiles of 512 tokens (2 w-rows x 256 h)
EPS = 1e-12
WSCALE = 64.0             # q,k fp8 weights are scaled by 2^6

# tap order: it = 3*k0 + k1;  DR pairs (0,1) (2,3) (4,5) (6,7) (8,zero):
# tap 9 is an all-zero weight column so the odd 9th tap still runs at the
# DoubleRow rate (its pair dim reads the same window twice, stride 0)
TAPS = [(k0, k1) for k0 in range(3) for k1 in range(3)]

_CACHE = {}


def _build(reps=0, hwloop=True):
    """Emit + compile the 8-core SPMD program. reps>0 wraps the compute in a
    hardware loop (used only for timing; collectives become local copies)."""
    nc = bacc.Bacc(None, target_bir_lowering=False, debug=False,
                   num_devices=NCORES)
    x_bf = nc.declare_dram_parameter("x_bf", [B, 128, WLH, HP], BF16,
                                     isOutput=False)
    x_f8 = nc.declare_dram_parameter("x_f8", [B, 128, WLH, HP], FP8,
                                     isOutput=False)
    w_v = nc.declare_dram_parameter("w_v", [128, 9, 128], BF16,
                                    isOutput=False)
    w_qk = nc.declare_dram_parameter("w_qk", [128, 2, 10, 128], FP8,
                                     isOutput=False)
    w_proj = nc.declare_dram_parameter("w_proj", [128, 128], BF16,
                                       isOutput=False)
    y_out = nc.declare_dram_parameter("y_out", [B, 128, WL, H], BF16,
                                      isOutput=True)

    with tile.TileContext(nc) as tc:
        with (
            tc.tile_pool(name="const", bufs=1) as const,
            tc.tile_pool(name="xp", bufs=1) as xp,
            tc.tile_pool(name="vp", bufs=1) as vp,
            tc.tile_pool(name="qks", bufs=3) as qks,
            tc.tile_pool(name="qkt", bufs=2) as qkt,
            tc.tile_pool(name="scr", bufs=2) as scr,
            tc.tile_pool(name="stat", bufs=1) as stat,
            tc.tile_pool(name="dram", bufs=1, space="DRAM") as dram,
            tc.tile_pool(name="psacc", bufs=1, space="PSUM") as psacc,
        ):
            wv = const.tile([128, 9, 128], BF16)
            nc.sync.dma_start(out=wv[:], in_=w_v[:])
            wqk = const.tile([128, 2, 10, 128], FP8)
            nc.sync.dma_start(out=wqk[:], in_=w_qk[:])
            wp = const.tile([128, 128], BF16)
            nc.sync.dma_start(out=wp[:], in_=w_proj[:])
            identf = const.tile([128, 128], F32)
            make_identity(nc, identf[:])
            identb = const.tile([128, 128], BF16)
            nc.vector.tensor_copy(out=identb[:], in_=identf[:])

            X = [xp.tile([128, WLH, HP], BF16, tag=f"x{b}", name=f"X{b}")
                 for b in range(B)]
            # fp8 copy kept flat with a 2-element guard: the last tile's
            # pad-position reads run 2 elements past the data end
            X8 = [xp.tile([128, WLH * HP + 2], FP8, tag=f"x8{b}",
                          name=f"X8{b}")
                  for b in range(B)]
            # front-load a small first chunk: the first conv tile only reads
            # w-rows 0:4, so a 4-row leading DMA halves time-to-first-matmul
            for b in range(B):
                for lo, hi in ((0, 4), (4, 12), (12, 20), (20, 27),
                               (27, WLH)):
                    nc.sync.dma_start(out=X[b][:, lo:hi, :],
                                      in_=x_bf[b, :, lo:hi, :])
                    x8v = X8[b][:, lo * HP:hi * HP].rearrange(
                        "p (w h) -> p w h", h=HP)
                    nc.sync.dma_start(out=x8v, in_=x_f8[b, :, lo:hi, :])
                nc.vector.memset(X8[b][:, WLH * HP:WLH * HP + 2], 0)

            V = [vp.tile([128, NWT, 512], BF16, tag=f"v{b}", name=f"V{b}")
                 for b in range(B)]
            # per-batch QK accumulator banks (double as softmax scratch)
            Ab = [psacc.tile([128, 256], F32, tag=f"acc{b}", name=f"A{b}")
                  for b in range(B)]
            kstats = stat.tile([128, B, NWT], F32)
            # S[c,o] = sum_m attn[m,c] * w_proj[m,o]: folding the projection
            # into the attention weights makes the tail one matmul per tile
            S = [stat.tile([128, 128], BF16, tag=f"S{b}", name=f"S{b}")
                 for b in range(B)]
            stats_s = [stat.tile([128, 130], F32, tag=f"st{b}", name=f"st{b}")
                       for b in range(B)]
            rstats = [stat.tile([128, 130], F32, tag=f"rst{b}", name=f"rst{b}")
                      for b in range(B)]
            cc_in = [dram.tile([128, 130], F32, tag=f"ci{b}", name=f"ci{b}")
                     for b in range(B)]
            cc_out = [dram.tile([128, 130], F32, tag=f"co{b}", name=f"co{b}")
                      for b in range(B)]

            def dr_rhs(xt, row, pair):
                """Overlapping AP [128, 2, 258] feeding a DoubleRow matmul:
                the input windows of taps pair=(tA,tB) for output w-row
                `row`, flat across one padded h-line.  Pair 4 = (8, zero):
                stride-0 pair dim rereads tap 8's window against the zero
                weight column."""
                k0a, k1a = TAPS[pair[0]]
                full = xt[:]
                pstride = full.ap[0][0]
                off = full.offset + (row + k0a) * HP + k1a
                if pair[1] < 9:
                    k0b, k1b = TAPS[pair[1]]
                    d = (k0b - k0a) * HP + (k1b - k1a)
                else:
                    d = 0
                return AP(full.tensor, off, [[pstride, 128], [d, 2],
                                             [1, HP]])

            loop_cm = (tc.For_i(0, reps, 1,
                                hint_engines=(mybir.EngineType.PE,
                                              mybir.EngineType.Activation,
                                              mybir.EngineType.DVE))
                       if reps and hwloop else None)
            if loop_cm is not None:
                loop_cm.__enter__()

            psconv_cm = tc.tile_pool(name="psconv", bufs=1, space="PSUM")
            pstr_cm = tc.tile_pool(name="pstr", bufs=1, space="PSUM")
            psconv = psconv_cm.__enter__()
            pstr = pstr_cm.__enter__()

            # per-tile work is software-pipelined over three stages so the
            # in-order PE queue never waits on ACT's PSUM->SBUF copies:
            #   stage 1 (wt):   conv matmuls -> pv/pq/pk; DVE copy to V,
            #                   ACT evictions q_s/k_s (fp8) + Square accums
            #   stage 2 (wt-1): 8 PE transposes of q_s/k_s into one packed
            #                   fp8 PSUM bank; ACT evictions into [kT|qT]
            #   stage 3 (wt-2): 2 DoubleRow QK^T accumulation matmuls
            qs_hist, ks_hist, qkt_hist = {}, {}, {}

            def conv_stage(b, wt):
                pv = psconv.tile([128, 512], F32, tag="pv", name="pv",
                                 bufs=1)
                # one PSUM bank per q/k output row (a padded 258-row doesn't
                # pack two to a bank); 3 rotating buffers for 4 rows/tile,
                # freed by early per-row evictions
                pq = [psconv.tile([128, 512], F32, tag="pqk",
                                  name=f"pq{r}", bufs=3) for r in range(2)]
                pk = [psconv.tile([128, 512], F32, tag="pqk",
                                  name=f"pk{r}", bufs=3) for r in range(2)]
                q_s = qks.tile([128, 512], BF16, tag="q_s", name="q_s")
                k_s = qks.tile([128, 512], BF16, tag="k_s", name="k_s")

                # interleave the three chunks' matmuls: every 256-col
                # DoubleRow weight load shadows under a 512-row v matmul,
                # and v's 128-col loads hide under the DR executions
                def v_mm(it):
                    k0, k1 = TAPS[it]
                    nc.tensor.matmul(
                        pv[:], wv[:, it, :],
                        X[b][:, 2 * wt + k0:2 * wt + k0 + 2, k1:k1 + 256],
                        start=(it == 0), stop=(it == 8))

                def qk_mm(pt, chunk, p):
                    for r in range(2):
                        nc.tensor.matmul(
                            pt[r][:, 0:HP],
                            wqk[:, chunk, 2 * p:2 * p + 2, :],
                            dr_rhs(X8[b], 2 * wt + r, (2 * p, 2 * p + 1)),
                            start=(p == 0), stop=(p == 4),
                            perf_mode=mybir.MatmulPerfMode.DoubleRow)

                def evict(dst, pt, scaled_dve):
                    # one row per engine: q rows on DVE, k rows on ACT
                    for r in range(2):
                        if scaled_dve:
                            nc.vector.tensor_scalar_mul(
                                out=dst[:, 256 * r:256 * (r + 1)],
                                in0=pt[r][:, 0:256], scalar1=1.0 / WSCALE)
                        else:
                            nc.scalar.activation(
                                out=dst[:, 256 * r:256 * (r + 1)],
                                in_=pt[r][:, 0:256],
                                func=mybir.ActivationFunctionType.Copy,
                                scale=1.0 / WSCALE)

                for i in range(5):
                    v_mm(i)
                    qk_mm(pq, 0, i)
                evict(q_s, pq, True)
                for i in range(5, 9):
                    v_mm(i)
                    qk_mm(pk, 1, i - 5)
                qk_mm(pk, 1, 4)
                nc.vector.tensor_copy(out=V[b][:, wt, :],
                                      in_=pv[:])
                evict(k_s, pk, False)
                sq_k = scr.tile([128, 512], F32, tag="sq_k", name="sq_k")
                nc.scalar.activation(
                    out=sq_k[:], in_=k_s[:],
                    func=mybir.ActivationFunctionType.Square,
                    accum_out=kstats[:, b, wt:wt + 1])
                qs_hist[wt], ks_hist[wt] = q_s, k_s

            def tr_stage(b, wt):
                # QKT block layout: [kT_j | qT_j] interleaved per 256 so the
                # QK DoubleRow rhs ([kT|qT] per token block) stays a 3D AP;
                # the wider rhs also restores QQ^T, whose diagonal provides
                # the q norms (no ACT Square pass needed for q)
                q_s, k_s = qs_hist.pop(wt), ks_hist.pop(wt)
                ptqk = pstr.tile([128, 1024], BF16, tag="ptqk", name="ptqk",
                                 bufs=2)
                for j in range(4):
                    sl = slice(128 * j, 128 * (j + 1))
                    nc.tensor.transpose(ptqk[:, 256 * j:256 * j + 128],
                                        k_s[:, sl], identb[:])
                    nc.tensor.transpose(ptqk[:, 256 * j + 128:256 * (j + 1)],
                                        q_s[:, sl], identb[:])
                QKT = qkt.tile([128, 1024], FP8, tag="QKT", name="QKT")
                nc.scalar.copy(out=QKT[:, 0:512], in_=ptqk[:, 0:512])
                nc.vector.tensor_copy(out=QKT[:, 512:1024],
                                      in_=ptqk[:, 512:1024])
                qkt_hist[wt] = QKT

            def qk_stage(b, wt):
                QKT = qkt_hist.pop(wt)
                blk = QKT[:].rearrange("p (j c) -> p j c", j=4)
                for j in (0, 2):
                    nc.tensor.matmul(
                        Ab[b][:, 0:256],
                        blk[:, j:j + 2, 128:256], blk[:, j:j + 2, :],
                        start=(wt == 0 and j == 0),
                        stop=(wt == NWT - 1 and j == 2),
                        perf_mode=mybir.MatmulPerfMode.DoubleRow)

            def conv_wt_pipelined(b, hooks=None):
                for step in range(NWT + 2):
                    if 1 <= step <= NWT:
                        tr_stage(b, step - 1)
                    if step < NWT:
                        conv_stage(b, step)
                    if step >= 2:
                        qk_stage(b, step - 2)
                    if hooks and step in hooks:
                        hooks[step]()

            def stats_and_cc(b):
                # local [QK | diag(QQ) | sum k^2] -> AllReduce across cores
                nc.scalar.copy(out=stats_s[b][:, 0:128], in_=Ab[b][:, 0:128])
                dscr = scr.tile([128, 128], F32, tag="dscr", name="dscr")
                nc.vector.tensor_tensor(out=dscr[:], in0=Ab[b][:, 128:256],
                                        in1=identf[:],
                                        op=mybir.AluOpType.mult)
                nc.vector.reduce_sum(out=stats_s[b][:, 128:129], in_=dscr[:],
                                     axis=mybir.AxisListType.X)
                nc.vector.reduce_sum(out=stats_s[b][:, 129:130],
                                     in_=kstats[:, b, :],
                                     axis=mybir.AxisListType.X)
                if reps:
                    nc.vector.tensor_copy(out=rstats[b][:], in_=stats_s[b][:])
                else:
                    nc.sync.dma_start(out=cc_in[b][:], in_=stats_s[b][:])
                    nc.gpsimd.collective_compute(
                        "AllReduce", mybir.AluOpType.add,
                        replica_groups=[list(range(NCORES))],
                        ins=[cc_in[b].opt()], outs=[cc_out[b].opt()],
                    )
                    nc.sync.dma_start(out=rstats[b][:], in_=cc_out[b][:])

            def softmax_s1(b, acc=False):
                # norms + first transpose + rk scaling.  acc=True uses
                # accumulate-mode PSUM writes (start=False): safe while the
                # other batch's QK still accumulates in the same bank, since
                # its start=True marked the whole bank pending (first write
                # per byte range overwrites)
                nq = scr.tile([128, 2], F32, tag="nq", name="nq")
                nc.scalar.activation(
                    out=nq[:], in_=rstats[b][:, 128:130],
                    func=mybir.ActivationFunctionType.Sqrt)
                nqm = scr.tile([128, 2], F32, tag="nqm", name="nqm")
                nc.vector.tensor_scalar_max(out=nqm[:], in0=nq[:],
                                            scalar1=EPS)
                rqk = scr.tile([128, 2], F32, tag="rqk", name="rqk")
                nc.vector.reciprocal(out=rqk[:], in_=nqm[:])
                nc.tensor.transpose(Ab[b][:, 0:128], rstats[b][:, 0:128],
                                    identf[:])
                s1 = scr.tile([128, 128], F32, tag="s1", name="s1")
                nc.scalar.copy(out=s1[:], in_=Ab[b][:, 0:128])
                s1b = scr.tile([128, 128], F32, tag="s1b", name="s1b")
                nc.vector.tensor_scalar_mul(out=s1b[:], in0=s1[:],
                                            scalar1=rqk[:, 1:2])
                return rqk, s1b

            def softmax_s2(b, rqk, s1b, acc=False):
                # second transpose + rq scaling + masked row softmax
                nc.tensor.transpose(Ab[b][:, 128:256], s1b[:], identf[:])
                lg = scr.tile([128, 128], F32, tag="lg", name="lg")
                nc.scalar.copy(out=lg[:], in_=Ab[b][:, 128:256])
                lg2 = scr.tile([128, 128], F32, tag="lg2", name="lg2")
                nc.vector.tensor_scalar_mul(out=lg2[:], in0=lg[:],
                                            scalar1=rqk[:, 0:1])
                nc.vector.memset(lg2[0:64, 64:128], -1e30)
                nc.vector.memset(lg2[64:128, 0:64], -1e30)
                mx = scr.tile([128, 1], F32, tag="mx", name="mx")
                nc.vector.reduce_max(out=mx[:], in_=lg2[:],
                                     axis=mybir.AxisListType.X)
                sh = scr.tile([128, 128], F32, tag="sh", name="sh")
                nc.vector.tensor_scalar(out=sh[:], in0=lg2[:], scalar1=mx[:],
                                        scalar2=None,
                                        op0=mybir.AluOpType.subtract)
                ex = scr.tile([128, 128], F32, tag="ex", name="ex")
                esum = scr.tile([128, 1], F32, tag="esum", name="esum")
                nc.scalar.activation(out=ex[:], in_=sh[:],
                                     func=mybir.ActivationFunctionType.Exp,
                                     accum_out=esum[:])
                rs = scr.tile([128, 1], F32, tag="rs", name="rs")
                nc.vector.reciprocal(out=rs[:], in_=esum[:])
                pr = scr.tile([128, 128], BF16, tag="pr", name="pr")
                nc.vector.tensor_scalar_mul(out=pr[:], in0=ex[:],
                                            scalar1=rs[:])
                return pr

            def softmax_s3(b, pr, acc=False):
                nc.tensor.matmul(Ab[b][:, 128:256], pr[:], wp[:],
                                 start=True, stop=True)
                nc.scalar.copy(out=S[b][:], in_=Ab[b][:, 128:256])

            # batch 0 conv; its stats exchange overlaps batch 1 conv, and
            # batch 0's whole softmax chain is woven into batch 1's conv
            # stream (the AllReduce result is ready a few tiles in)
            sm0 = {}

            def sm0_s1():
                sm0['st'] = softmax_s1(0, acc=True)

            def sm0_s2():
                sm0['pr'] = softmax_s2(0, *sm0['st'], acc=True)

            def sm0_s3():
                softmax_s3(0, sm0['pr'], acc=True)

            conv_wt_pipelined(0)
            stats_and_cc(0)
            conv_wt_pipelined(1, hooks={8: sm0_s1, 11: sm0_s2, 14: sm0_s3})
            stats_and_cc(1)

            pstr_cm.__exit__(None, None, None)
            psconv_cm.__exit__(None, None, None)

            # ---- fused (attn.proj) @ v, writeback ----
            # batch 0's tail starts immediately (S[0] is ready), hiding
            # batch 1's stats AllReduce; batch 1's softmax stages weave
            # between batch 0's tail tiles.  y DMAs batch 4 tiles each.
            with tc.tile_pool(name="psout", bufs=4, space="PSUM") as psout:
                y3 = y_out[:].rearrange("b p w h -> b p (w h)")

                def tail_tile(b, wt):
                    pout = psout.tile([128, 512], F32, tag="pout",
                                      name="pout")
                    nc.tensor.matmul(pout[:], S[b][:], V[b][:, wt, :],
                                     start=True, stop=True)
                    if wt % 4 == 0:
                        sm0['ysb'] = scr.tile([128, 4, 512], BF16,
                                              tag="ysb", name="ysb", bufs=2)
                    ysb = sm0['ysb']
                    # split each eviction across both engines so neither
                    # is the tail's throughput limit
                    nc.scalar.copy(out=ysb[:, wt % 4, 0:256],
                                   in_=pout[:, 0:256])
                    nc.vector.tensor_copy(out=ysb[:, wt % 4, 256:512],
                                          in_=pout[:, 256:512])
                    if wt % 4 == 3:
                        g = wt // 4
                        nc.sync.dma_start(
                            out=y3[b, :, 2048 * g:2048 * (g + 1)],
                            in_=ysb[:])

                sm1 = {}
                hooks1 = {
                    2: lambda: sm1.__setitem__('st', softmax_s1(1)),
                    6: lambda: sm1.__setitem__('pr',
                                               softmax_s2(1, *sm1['st'])),
                    10: lambda: softmax_s3(1, sm1['pr']),
                }
                for wt in range(NWT):
                    tail_tile(0, wt)
                    if wt in hooks1:
                        hooks1[wt]()
                for wt in range(NWT):
                    tail_tile(1, wt)

            if loop_cm is not None:
                loop_cm.__exit__(None, None, None)

    nc.compile()
    return nc


def _get_nc(reps=0, hwloop=True):
    key = (reps, hwloop)
    if key not in _CACHE:
        _CACHE[key] = _build(reps, hwloop)
    return _CACHE[key]


def _prep_inputs(x, qkv_wr, qkv_wi, dw_wr, dw_wi, proj_wr, proj_wi):
    import ml_dtypes
    bf16 = ml_dtypes.bfloat16
    f8 = mybir.dt.np(FP8)

    cw = np.complex128
    Q = (qkv_wr[:, :, 0, 0].astype(np.float64)
         + 1j * qkv_wi[:, :, 0, 0].astype(np.float64))
    D = (dw_wr[:, 0].astype(np.float64)
         + 1j * dw_wi[:, 0].astype(np.float64)).reshape(3 * C, 9).astype(cw)
    w_full = np.zeros((128, 9, 3, 128), np.float32)
    for t in range(9):
        F = D[:, t:t + 1] * Q            # [192, 64] complex
        for chunk in range(3):
            Fc = F[64 * chunk:64 * (chunk + 1)]   # [64 out, 64 in]
            Re, Im = Fc.real.T, Fc.imag.T          # [in, out]
            w_full[:, t, chunk, :] = np.block([[Re, Im], [-Im, Re]])
    # chunk order in w_full: 0=q, 1=k, 2=v
    w_v = np.ascontiguousarray(w_full[:, :, 2, :]).astype(bf16)
    w_qk9 = np.ascontiguousarray(
        w_full[:, :, 0:2, :].transpose(0, 2, 1, 3)) * WSCALE   # [128,2,9,128]
    w_qk = np.zeros((128, 2, 10, 128), np.float32)
    w_qk[:, :, 0:9, :] = w_qk9
    w_qk = w_qk.astype(f8)
    P = (proj_wr[:, :, 0, 0].astype(np.float64)
         + 1j * proj_wi[:, :, 0, 0].astype(np.float64))
    Re, Im = P.real.T, P.imag.T
    w_proj = np.block([[Re, Im], [-Im, Re]]).astype(bf16)

    xpad = np.pad(np.asarray(x, np.float32),
                  ((0, 0), (0, 0), (1, 1), (0, 0), (0, 0)))
    in_maps = []
    for core in range(NCORES):
        xs = xpad[:, :, WL * core:WL * core + WLH, :, :]
        xc = np.zeros((B, 128, WLH, HP), np.float32)
        xc[:, :C, :, 1:H + 1] = xs[..., 0]
        xc[:, C:, :, 1:H + 1] = xs[..., 1]
        in_maps.append({"x_bf": xc.astype(bf16), "x_f8": xc.astype(f8),
                        "w_v": w_v, "w_qk": w_qk, "w_proj": w_proj})
    return in_maps


def _assemble(results):
    out = np.empty((B, C, W, H, 2), np.float32)
    for core in range(NCORES):
        o = results[core]["y_out"].astype(np.float32).reshape(B, 2, C, WL, H)
        out[:, :, WL * core:WL * (core + 1), :, :] = o.transpose(0, 2, 3, 4, 1)
    return out


def kernel(x, qkv_wr, qkv_wi, dw_wr, dw_wi, proj_wr, proj_wi):
    nc = _get_nc()
    in_maps = _prep_inputs(x, qkv_wr, qkv_wi, dw_wr, dw_wi,
                           proj_wr, proj_wi)
    res = run_bass_kernel_spmd(nc, in_maps, list(range(NCORES)))
    return _assemble(res.results)



# revision 2
# speedup vs baseline: 1.6348x; 1.6348x over previous
"""Trainium2 Bass kernel for complex-valued channel attention (XCA-style) — v3.

Same algorithm/dtype tiering as v2 (fused 9-tap dense conv: v in bf16,
q/k in fp8 DoubleRow; QK^T in fp8 DR over transposed token blocks;
attn+proj folded into S; bf16 writeback), plus scheduling fixes found via
cost-model timeline simulation:

  - softmax split into six fine-grained stages, hooked into the conv/tail
    streams so each PE op (transposes, projection matmul) is issued one
    hook AFTER the V-engine ops it depends on: the in-order PE queue no
    longer stalls 1-2.3us behind same-hook DVE/ACT chains that are queued
    after a full pipeline step of conv evictions.
  - tail y DMAs issued in half-groups, and per-tile for the final group:
    the end-of-kernel drain is one small DMA instead of a 4-tile burst.
  - finer input-DMA chunking so the first conv tile starts sooner and the
    ramp stays ahead of compute.
"""

import sys

sys.path.insert(0, '/opt/trn_rl_repo')

import numpy as np

import concourse.bass as bass  # noqa: F401  (registers bass types)
import concourse.tile as tile
from concourse import bacc, mybir
from concourse.ap import AP
from concourse.bass_utils import run_bass_kernel_spmd
from concourse.masks import make_identity

F32 = mybir.dt.float32
BF16 = mybir.dt.bfloat16
FP8 = mybir.dt.float8e4

B, C, W, H = 2, 64, 256, 256
NCORES = 8
WL = W // NCORES          # 32 local w rows per core
HP = H + 2                # 258: h with zero pad columns
WLH = WL + 2              # 34: local w rows + halo
NWT = WL // 2             # 16 tiles of 512 tokens (2 w-rows x 256 h)
EPS = 1e-12
WSCALE = 64.0             # q,k fp8 weights are scaled by 2^6

# tap order: it = 3*k0 + k1;  DR pairs (0,1) (2,3) (4,5) (6,7) (8,zero):
# tap 9 is an all-zero weight column so the odd 9th tap still runs at the
# DoubleRow rate (its pair dim reads the same window twice, stride 0)
TAPS = [(k0, k1) for k0 in range(3) for k1 in range(3)]

_CACHE = {}


def _build(reps=0, hwloop=True):
    """Emit + compile the 8-core SPMD program. reps>0 wraps the compute in a
    hardware loop (used only for timing; collectives become local copies)."""
    nc = bacc.Bacc(None, target_bir_lowering=False, debug=False,
                   num_devices=NCORES)
    x_bf = nc.declare_dram_parameter("x_bf", [B, 128, WLH, HP], BF16,
                                     isOutput=False)
    x_f8 = nc.declare_dram_parameter("x_f8", [B, 128, WLH, HP], FP8,
                                     isOutput=False)
    w_v = nc.declare_dram_parameter("w_v", [128, 9, 128], BF16,
                                    isOutput=False)
    w_qk = nc.declare_dram_parameter("w_qk", [128, 2, 10, 128], FP8,
                                     isOutput=False)
    w_proj = nc.declare_dram_parameter("w_proj", [128, 128], BF16,
                                       isOutput=False)
    y_out = nc.declare_dram_parameter("y_out", [B, 128, WL, H], BF16,
                                      isOutput=True)

    with tile.TileContext(nc) as tc:
        with (
            tc.tile_pool(name="const", bufs=1) as const,
            tc.tile_pool(name="xp", bufs=1) as xp,
            tc.tile_pool(name="vp", bufs=1) as vp,
            tc.tile_pool(name="qks", bufs=3) as qks,
            tc.tile_pool(name="qkt", bufs=2) as qkt,
            tc.tile_pool(name="scr", bufs=2) as scr,
            tc.tile_pool(name="stat", bufs=1) as stat,
            tc.tile_pool(name="dram", bufs=1, space="DRAM") as dram,
            tc.tile_pool(name="psacc", bufs=1, space="PSUM") as psacc,
        ):
            wv = const.tile([128, 9, 128], BF16)
            nc.sync.dma_start(out=wv[:], in_=w_v[:])
            wqk = const.tile([128, 2, 10, 128], FP8)
            nc.sync.dma_start(out=wqk[:], in_=w_qk[:])
            wp = const.tile([128, 128], BF16)
            nc.sync.dma_start(out=wp[:], in_=w_proj[:])
            identf = const.tile([128, 128], F32)
            make_identity(nc, identf[:])
            identb = const.tile([128, 128], BF16)
            nc.vector.tensor_copy(out=identb[:], in_=identf[:])

            X = [xp.tile([128, WLH, HP], BF16, tag=f"x{b}", name=f"X{b}")
                 for b in range(B)]
            # fp8 copy kept flat with a 2-element guard: the last tile's
            # pad-position reads run 2 elements past the data end
            X8 = [xp.tile([128, WLH * HP + 2], FP8, tag=f"x8{b}",
                          name=f"X8{b}")
                  for b in range(B)]
            # front-load small first chunks: conv tile t reads w-rows
            # 2t..2t+3, so a fine-grained leading ramp minimizes
            # time-to-first-matmul and keeps the ramp ahead of compute
            for b in range(B):
                for lo, hi in ((0, 4), (4, 8), (8, 14), (14, 21),
                               (21, 28), (28, WLH)):
                    nc.sync.dma_start(out=X[b][:, lo:hi, :],
                                      in_=x_bf[b, :, lo:hi, :])
                    x8v = X8[b][:, lo * HP:hi * HP].rearrange(
                        "p (w h) -> p w h", h=HP)
                    nc.sync.dma_start(out=x8v, in_=x_f8[b, :, lo:hi, :])
                nc.vector.memset(X8[b][:, WLH * HP:WLH * HP + 2], 0)

            V = [vp.tile([128, NWT, 512], BF16, tag=f"v{b}", name=f"V{b}")
                 for b in range(B)]
            # per-batch QK accumulator banks (double as softmax scratch)
            Ab = [psacc.tile([128, 256], F32, tag=f"acc{b}", name=f"A{b}")
                  for b in range(B)]
            kstats = stat.tile([128, B, NWT], F32)
            # S[c,o] = sum_m attn[m,c] * w_proj[m,o]: folding the projection
            # into the attention weights makes the tail one matmul per tile
            S = [stat.tile([128, 128], BF16, tag=f"S{b}", name=f"S{b}")
                 for b in range(B)]
            stats_s = [stat.tile([128, 130], F32, tag=f"st{b}", name=f"st{b}")
                       for b in range(B)]
            rstats = [stat.tile([128, 130], F32, tag=f"rst{b}", name=f"rst{b}")
                      for b in range(B)]
            cc_in = [dram.tile([128, 130], F32, tag=f"ci{b}", name=f"ci{b}")
                     for b in range(B)]
            cc_out = [dram.tile([128, 130], F32, tag=f"co{b}", name=f"co{b}")
                      for b in range(B)]

            def dr_rhs(xt, row, pair):
                """Overlapping AP [128, 2, 258] feeding a DoubleRow matmul:
                the input windows of taps pair=(tA,tB) for output w-row
                `row`, flat across one padded h-line.  Pair 4 = (8, zero):
                stride-0 pair dim rereads tap 8's window against the zero
                weight column."""
                k0a, k1a = TAPS[pair[0]]
                full = xt[:]
                pstride = full.ap[0][0]
                off = full.offset + (row + k0a) * HP + k1a
                if pair[1] < 9:
                    k0b, k1b = TAPS[pair[1]]
                    d = (k0b - k0a) * HP + (k1b - k1a)
                else:
                    d = 0
                return AP(full.tensor, off, [[pstride, 128], [d, 2],
                                             [1, HP]])

            loop_cm = (tc.For_i(0, reps, 1,
                                hint_engines=(mybir.EngineType.PE,
                                              mybir.EngineType.Activation,
                                              mybir.EngineType.DVE))
                       if reps and hwloop else None)
            if loop_cm is not None:
                loop_cm.__enter__()

            psconv_cm = tc.tile_pool(name="psconv", bufs=1, space="PSUM")
            pstr_cm = tc.tile_pool(name="pstr", bufs=1, space="PSUM")
            psconv = psconv_cm.__enter__()
            pstr = pstr_cm.__enter__()

            # per-tile work is software-pipelined over three stages so the
            # in-order PE queue never waits on ACT's PSUM->SBUF copies:
            #   stage 1 (wt):   conv matmuls -> pv/pq/pk; DVE copy to V,
            #                   ACT evictions q_s/k_s (fp8) + Square accums
            #   stage 2 (wt-1): 8 PE transposes of q_s/k_s into one packed
            #                   fp8 PSUM bank; ACT evictions into [kT|qT]
            #   stage 3 (wt-2): 2 DoubleRow QK^T accumulation matmuls
            qs_hist, ks_hist, qkt_hist = {}, {}, {}

            def conv_stage(b, wt):
                pv = psconv.tile([128, 512], F32, tag="pv", name="pv",
                                 bufs=1)
                # one PSUM bank per q/k output row (a padded 258-row doesn't
                # pack two to a bank); 3 rotating buffers for 4 rows/tile,
                # freed by early per-row evictions
                pq = [psconv.tile([128, 512], F32, tag="pqk",
                                  name=f"pq{r}", bufs=3) for r in range(2)]
                pk = [psconv.tile([128, 512], F32, tag="pqk",
                                  name=f"pk{r}", bufs=3) for r in range(2)]
                q_s = qks.tile([128, 512], BF16, tag="q_s", name="q_s")
                k_s = qks.tile([128, 512], BF16, tag="k_s", name="k_s")

                # interleave the three chunks' matmuls: every 256-col
                # DoubleRow weight load shadows under a 512-row v matmul,
                # and v's 128-col loads hide under the DR executions
                def v_mm(it):
                    k0, k1 = TAPS[it]
                    nc.tensor.matmul(
                        pv[:], wv[:, it, :],
                        X[b][:, 2 * wt + k0:2 * wt + k0 + 2, k1:k1 + 256],
                        start=(it == 0), stop=(it == 8))

                def qk_mm(pt, chunk, p):
                    for r in range(2):
                        nc.tensor.matmul(
                            pt[r][:, 0:HP],
                            wqk[:, chunk, 2 * p:2 * p + 2, :],
                            dr_rhs(X8[b], 2 * wt + r, (2 * p, 2 * p + 1)),
                            start=(p == 0), stop=(p == 4),
                            perf_mode=mybir.MatmulPerfMode.DoubleRow)

                def evict(dst, pt, scaled_dve):
                    # one row per engine: q rows on DVE, k rows on ACT
                    for r in range(2):
                        if scaled_dve:
                            nc.vector.tensor_scalar_mul(
                                out=dst[:, 256 * r:256 * (r + 1)],
                                in0=pt[r][:, 0:256], scalar1=1.0 / WSCALE)
                        else:
                            nc.scalar.activation(
                                out=dst[:, 256 * r:256 * (r + 1)],
                                in_=pt[r][:, 0:256],
                                func=mybir.ActivationFunctionType.Copy,
                                scale=1.0 / WSCALE)

                for i in range(5):
                    v_mm(i)
                    qk_mm(pq, 0, i)
                evict(q_s, pq, True)
                for i in range(5, 9):
                    v_mm(i)
                    qk_mm(pk, 1, i - 5)
                qk_mm(pk, 1, 4)
                nc.vector.tensor_copy(out=V[b][:, wt, :],
                                      in_=pv[:])
                evict(k_s, pk, False)
                sq_k = scr.tile([128, 512], F32, tag="sq_k", name="sq_k")
                nc.scalar.activation(
                    out=sq_k[:], in_=k_s[:],
                    func=mybir.ActivationFunctionType.Square,
                    accum_out=kstats[:, b, wt:wt + 1])
                qs_hist[wt], ks_hist[wt] = q_s, k_s

            def tr_stage(b, wt):
                # QKT block layout: [kT_j | qT_j] interleaved per 256 so the
                # QK DoubleRow rhs ([kT|qT] per token block) stays a 3D AP;
                # the wider rhs also restores QQ^T, whose diagonal provides
                # the q norms (no ACT Square pass needed for q)
                q_s, k_s = qs_hist.pop(wt), ks_hist.pop(wt)
                ptqk = pstr.tile([128, 1024], BF16, tag="ptqk", name="ptqk",
                                 bufs=2)
                for j in range(4):
                    sl = slice(128 * j, 128 * (j + 1))
                    nc.tensor.transpose(ptqk[:, 256 * j:256 * j + 128],
                                        k_s[:, sl], identb[:])
                    nc.tensor.transpose(ptqk[:, 256 * j + 128:256 * (j + 1)],
                                        q_s[:, sl], identb[:])
                QKT = qkt.tile([128, 1024], FP8, tag="QKT", name="QKT")
                nc.scalar.copy(out=QKT[:, 0:512], in_=ptqk[:, 0:512])
                nc.vector.tensor_copy(out=QKT[:, 512:1024],
                                      in_=ptqk[:, 512:1024])
                qkt_hist[wt] = QKT

            def qk_stage(b, wt):
                QKT = qkt_hist.pop(wt)
                blk = QKT[:].rearrange("p (j c) -> p j c", j=4)
                for j in (0, 2):
                    nc.tensor.matmul(
                        Ab[b][:, 0:256],
                        blk[:, j:j + 2, 128:256], blk[:, j:j + 2, :],
                        start=(wt == 0 and j == 0),
                        stop=(wt == NWT - 1 and j == 2),
                        perf_mode=mybir.MatmulPerfMode.DoubleRow)

            def conv_wt_pipelined(b, hooks=None):
                for step in range(NWT + 2):
                    if 1 <= step <= NWT:
                        tr_stage(b, step - 1)
                    if step < NWT:
                        conv_stage(b, step)
                    if step >= 2:
                        qk_stage(b, step - 2)
                    if hooks and step in hooks:
                        hooks[step]()

            def stats_and_cc(b):
                # local [QK | diag(QQ) | sum k^2] -> AllReduce across cores
                nc.scalar.copy(out=stats_s[b][:, 0:128], in_=Ab[b][:, 0:128])
                dscr = scr.tile([128, 128], F32, tag="dscr", name="dscr")
                nc.vector.tensor_tensor(out=dscr[:], in0=Ab[b][:, 128:256],
                                        in1=identf[:],
                                        op=mybir.AluOpType.mult)
                nc.vector.reduce_sum(out=stats_s[b][:, 128:129], in_=dscr[:],
                                     axis=mybir.AxisListType.X)
                nc.vector.reduce_sum(out=stats_s[b][:, 129:130],
                                     in_=kstats[:, b, :],
                                     axis=mybir.AxisListType.X)
                if reps:
                    nc.vector.tensor_copy(out=rstats[b][:], in_=stats_s[b][:])
                else:
                    nc.sync.dma_start(out=cc_in[b][:], in_=stats_s[b][:])
                    nc.gpsimd.collective_compute(
                        "AllReduce", mybir.AluOpType.add,
                        replica_groups=[list(range(NCORES))],
                        ins=[cc_in[b].opt()], outs=[cc_out[b].opt()],
                    )
                    nc.sync.dma_start(out=rstats[b][:], in_=cc_out[b][:])

            # softmax split into fine-grained stages so each PE op (the two
            # transposes and the projection matmul) is issued one hook AFTER
            # the V-engine ops it depends on: the in-order PE queue then
            # never stalls behind a same-hook DVE/ACT chain that is itself
            # queued behind a full pipeline step of conv evictions.
            # PSUM writes into Ab use accumulate-mode semantics as before:
            # while the other batch's QK still accumulates in the same pool,
            # the first write per byte range overwrites.
            def sm_a(b, st):
                # V-engine only: reciprocal norms
                nq = scr.tile([128, 2], F32, tag="nq", name="nq")
                nc.scalar.activation(
                    out=nq[:], in_=rstats[b][:, 128:130],
                    func=mybir.ActivationFunctionType.Sqrt)
                nqm = scr.tile([128, 2], F32, tag="nqm", name="nqm")
                nc.vector.tensor_scalar_max(out=nqm[:], in0=nq[:],
                                            scalar1=EPS)
                rqk = scr.tile([128, 2], F32, tag="rqk", name="rqk",
                               bufs=2)
                nc.vector.reciprocal(out=rqk[:], in_=nqm[:])
                st['rqk'] = rqk

            def sm_b(b, st):
                # PE transpose 1 (depends only on rstats) + ACT eviction
                nc.tensor.transpose(Ab[b][:, 0:128], rstats[b][:, 0:128],
                                    identf[:])
                s1 = scr.tile([128, 128], F32, tag="s1", name="s1", bufs=2)
                nc.scalar.copy(out=s1[:], in_=Ab[b][:, 0:128])
                st['s1'] = s1

            def sm_c(b, st):
                # DVE rk scaling
                s1b = scr.tile([128, 128], F32, tag="s1b", name="s1b",
                               bufs=2)
                nc.vector.tensor_scalar_mul(out=s1b[:], in0=st['s1'][:],
                                            scalar1=st['rqk'][:, 1:2])
                st['s1b'] = s1b

            def sm_d(b, st):
                # PE transpose 2 + ACT eviction
                nc.tensor.transpose(Ab[b][:, 128:256], st['s1b'][:],
                                    identf[:])
                lg = scr.tile([128, 128], F32, tag="lg", name="lg", bufs=2)
                nc.scalar.copy(out=lg[:], in_=Ab[b][:, 128:256])
                st['lg'] = lg

            def sm_e(b, st):
                # rq scaling + masked row softmax (V-engines only)
                lg2 = scr.tile([128, 128], F32, tag="lg2", name="lg2")
                nc.vector.tensor_scalar_mul(out=lg2[:], in0=st['lg'][:],
                                            scalar1=st['rqk'][:, 0:1])
                nc.vector.memset(lg2[0:64, 64:128], -1e30)
                nc.vector.memset(lg2[64:128, 0:64], -1e30)
                mx = scr.tile([128, 1], F32, tag="mx", name="mx")
                nc.vector.reduce_max(out=mx[:], in_=lg2[:],
                                     axis=mybir.AxisListType.X)
                sh = scr.tile([128, 128], F32, tag="sh", name="sh")
                nc.vector.tensor_scalar(out=sh[:], in0=lg2[:], scalar1=mx[:],
                                        scalar2=None,
                                        op0=mybir.AluOpType.subtract)
                ex = scr.tile([128, 128], F32, tag="ex", name="ex")
                esum = scr.tile([128, 1], F32, tag="esum", name="esum")
                nc.scalar.activation(out=ex[:], in_=sh[:],
                                     func=mybir.ActivationFunctionType.Exp,
                                     accum_out=esum[:])
                rs = scr.tile([128, 1], F32, tag="rs", name="rs")
                nc.vector.reciprocal(out=rs[:], in_=esum[:])
                pr = scr.tile([128, 128], BF16, tag="pr", name="pr",
                              bufs=2)
                nc.vector.tensor_scalar_mul(out=pr[:], in0=ex[:],
                                            scalar1=rs[:])
                st['pr'] = pr

            def sm_f(b, st):
                # PE projection fold + ACT eviction into S
                nc.tensor.matmul(Ab[b][:, 128:256], st['pr'][:], wp[:],
                                 start=True, stop=True)
                nc.scalar.copy(out=S[b][:], in_=Ab[b][:, 128:256])

            SM_STAGES = (sm_a, sm_b, sm_c, sm_d, sm_e, sm_f)

            # batch 0 conv; its stats exchange overlaps batch 1 conv, and
            # batch 0's whole softmax chain is woven into batch 1's conv
            # stream (the AllReduce result is ready a few tiles in)
            sm0 = {}
            conv_wt_pipelined(0)
            stats_and_cc(0)
            hooks0 = {6 + 2 * i: (lambda f=f: f(0, sm0))
                      for i, f in enumerate(SM_STAGES)}
            conv_wt_pipelined(1, hooks=hooks0)
            stats_and_cc(1)

            pstr_cm.__exit__(None, None, None)
            psconv_cm.__exit__(None, None, None)

            # ---- fused (attn.proj) @ v, writeback ----
            # batch 0's tail starts immediately (S[0] is ready), hiding
            # batch 1's stats AllReduce; batch 1's softmax stages weave
            # between batch 0's tail tiles.  y DMAs batch 4 tiles each.
            with tc.tile_pool(name="psout", bufs=4, space="PSUM") as psout:
                y3 = y_out[:].rearrange("b p w h -> b p (w h)")

                def tail_tile(b, wt):
                    pout = psout.tile([128, 512], F32, tag="pout",
                                      name="pout")
                    nc.tensor.matmul(pout[:], S[b][:], V[b][:, wt, :],
                                     start=True, stop=True)
                    if wt % 4 == 0:
                        sm0['ysb'] = scr.tile([128, 4, 512], BF16,
                                              tag="ysb", name="ysb", bufs=3)
                    ysb = sm0['ysb']
                    # split each eviction across both engines so neither
                    # is the tail's throughput limit
                    nc.scalar.copy(out=ysb[:, wt % 4, 0:256],
                                   in_=pout[:, 0:256])
                    nc.vector.tensor_copy(out=ysb[:, wt % 4, 256:512],
                                          in_=pout[:, 256:512])
                    # half-group DMAs drain the staging buffer earlier; the
                    # final group goes out per-tile so the end-of-kernel
                    # drain is one small DMA, not a 4-tile burst
                    last_group = (b == B - 1 and wt >= NWT - 4)
                    if last_group:
                        nc.sync.dma_start(
                            out=y3[b, :, 512 * wt:512 * (wt + 1)],
                            in_=ysb[:, wt % 4, :])
                    elif wt % 4 == 1:
                        g = wt // 4
                        nc.sync.dma_start(
                            out=y3[b, :, 2048 * g:2048 * g + 1024],
                            in_=ysb[:, 0:2, :])
                    elif wt % 4 == 3:
                        g = wt // 4
                        nc.sync.dma_start(
                            out=y3[b, :, 2048 * g + 1024:2048 * (g + 1)],
                            in_=ysb[:, 2:4, :])

                sm1 = {}
                hooks1 = {k: (lambda f=f: f(1, sm1))
                          for k, f in zip((1, 2, 3, 4, 5, 7), SM_STAGES)}
                for wt in range(NWT):
                    tail_tile(0, wt)
                    if wt in hooks1:
                        hooks1[wt]()
                for wt in range(NWT):
                    tail_tile(1, wt)

            if loop_cm is not None:
                loop_cm.__exit__(None, None, None)

    nc.compile()
    return nc


def _get_nc(reps=0, hwloop=True):
    key = (reps, hwloop)
    if key not in _CACHE:
        _CACHE[key] = _build(reps, hwloop)
    return _CACHE[key]


def _prep_inputs(x, qkv_wr, qkv_wi, dw_wr, dw_wi, proj_wr, proj_wi):
    import ml_dtypes
    bf16 = ml_dtypes.bfloat16
    f8 = mybir.dt.np(FP8)

    cw = np.complex128
    Q = (qkv_wr[:, :, 0, 0].astype(np.float64)
         + 1j * qkv_wi[:, :, 0, 0].astype(np.float64))
    D = (dw_wr[:, 0].astype(np.float64)
         + 1j * dw_wi[:, 0].astype(np.float64)).reshape(3 * C, 9).astype(cw)
    w_full = np.zeros((128, 9, 3, 128), np.float32)
    for t in range(9):
        F = D[:, t:t + 1] * Q            # [192, 64] complex
        for chunk in range(3):
            Fc = F[64 * chunk:64 * (chunk + 1)]   # [64 out, 64 in]
            Re, Im = Fc.real.T, Fc.imag.T          # [in, out]
            w_full[:, t, chunk, :] = np.block([[Re, Im], [-Im, Re]])
    # chunk order in w_full: 0=q, 1=k, 2=v
    w_v = np.ascontiguousarray(w_full[:, :, 2, :]).astype(bf16)
    w_qk9 = np.ascontiguousarray(
        w_full[:, :, 0:2, :].transpose(0, 2, 1, 3)) * WSCALE   # [128,2,9,128]
    w_qk = np.zeros((128, 2, 10, 128), np.float32)
    w_qk[:, :, 0:9, :] = w_qk9
    w_qk = w_qk.astype(f8)
    P = (proj_wr[:, :, 0, 0].astype(np.float64)
         + 1j * proj_wi[:, :, 0, 0].astype(np.float64))
    Re, Im = P.real.T, P.imag.T
    w_proj = np.block([[Re, Im], [-Im, Re]]).astype(bf16)

    xpad = np.pad(np.asarray(x, np.float32),
                  ((0, 0), (0, 0), (1, 1), (0, 0), (0, 0)))
    in_maps = []
    for core in range(NCORES):
        xs = xpad[:, :, WL * core:WL * core + WLH, :, :]
        xc = np.zeros((B, 128, WLH, HP), np.float32)
        xc[:, :C, :, 1:H + 1] = xs[..., 0]
        xc[:, C:, :, 1:H + 1] = xs[..., 1]
        in_maps.append({"x_bf": xc.astype(bf16), "x_f8": xc.astype(f8),
                        "w_v": w_v, "w_qk": w_qk, "w_proj": w_proj})
    return in_maps


def _assemble(results):
    out = np.empty((B, C, W, H, 2), np.float32)
    for core in range(NCORES):
        o = results[core]["y_out"].astype(np.float32).reshape(B, 2, C, WL, H)
        out[:, :, WL * core:WL * (core + 1), :, :] = o.transpose(0, 2, 3, 4, 1)
    return out


def kernel(x, qkv_wr, qkv_wi, dw_wr, dw_wi, proj_wr, proj_wi):
    nc = _get_nc()
    in_maps = _prep_inputs(x, qkv_wr, qkv_wi, dw_wr, dw_wi,
                           proj_wr, proj_wi)
    res = run_bass_kernel_spmd(nc, in_maps, list(range(NCORES)))
    return _assemble(res.results)

